# revision 18
# baseline (speedup 1.0000x reference)
"""CAM (channel attention module) Trainium2 Bass kernel.

Reference computation (per sample, x: [C, N] with N = H*W):
    energy    = x @ x.T                      # [C, C] Gram matrix
    att       = softmax(rowmax(energy) - energy, axis=-1)
              = softmax(-energy, axis=-1)    # identical after max-shift
    out       = att @ x                      # [C, N]
    result    = gamma * out + x

Sharding: data-parallel over batch, B=16 -> 2 samples per core on 8 cores.

The kernel is HBM-bound: 16 MiB in + 16 MiB out per sample, 64 MiB per
core against the ~360 B/ns DMA_ENGINES roofline = 186.4 us.  The design
goal is to keep the DMA resource 100% occupied:

  - x is loaded in [128, 1024] f32 tiles (4 KiB descriptors) through a
    small staging pool and immediately downcast to a RESIDENT bf16 copy
    (8 MiB/sample instead of 16 MiB f32), alternating Pool/DVE so the
    downcast rate beats the load rate.  Halved residency means the next
    sample's loads are never buffer-starved: the DMA queue always has
    either loads or stores available.
  - Phase 1 transposes run on bf16 (1 cyc/row vs 2 for f32) so the PE
    keeps pace with the load stream; two 128-col transposes batch into
    one [128, 512] PSUM tile and a single ScalarE eviction (the ACT
    fixed overhead is ~185 ns/instruction).
  - The energy Gram matrix accumulates in PSUM f32 from bf16 operands;
    softmax is the min-shift rewrite of the reference's max-shifted
    form, with the 1/denom and gamma folded into one per-row scalar.
  - Phase 2 computes gamma/denom * (attT.T @ x_bf16) + x_bf16 per
    [128, 512] PSUM chunk, pairing chunks into [128, 1024] stores.
    The +x residual uses the bf16 copy (rel-err ~1e-3, far inside the
    2e-2 gate); with the reference's gamma == 0 the attention term
    vanishes entirely and the output is bf16-rounded x.
  - Sample s+1's loads are emitted ahead of sample s's stores each
    column so the 8 round-robin HW DMA lanes never head-of-line block
    on a store that waits for softmax.
"""

import threading

import numpy as np

import concourse.bass as bass
import concourse.mybir as mybir
import concourse.tile as tile
from concourse import bacc
from concourse.bass_utils import run_bass_kernel_spmd
from concourse.masks import make_identity

P = 128
F32 = mybir.dt.float32
BF16 = mybir.dt.bfloat16

# Full-problem shapes (hardcoded per harness contract).
B_FULL = 16
C_FULL = 256
H_FULL = W_FULL = 128
N_CORES = 8
B_PER_CORE = B_FULL // N_CORES  # 2


def emit_cam(tc, x, gamma_b, out, n_s, C, N, w_col=2048, ch=512,
             xf32_bufs=4, xfb_extra=0, xft_bufs=4, osb_bufs=3,
             ptr_bufs=3, pout_bufs=2, eps_bufs=2, ptr2_bufs=1,
             burst_cols=4, hold_cols=1,
             dc_engines=("gpsimd", "vector")):
    """Emit the per-core CAM kernel.

    x:       DRAM [n_s, C, N] f32
    gamma_b: DRAM [128, 1] f32 (gamma broadcast to all partitions on host)
    out:     DRAM [n_s, C, N] f32
    """
    nc = tc.nc
    cb_n = C // P                 # channel blocks (2)
    ncols = N // w_col            # load/store columns per sample (16)
    tpc = w_col // P              # transpose tiles per column (8)
    gpc = tpc // 4                # transpose groups (of 4 tiles) per column (2)
    ngr = N // (4 * P)            # transpose groups per sample (32)
    hpc = w_col // ch             # phase-2 psum chunks per column (2)
    assert C == 2 * P and w_col % (4 * P) == 0 and w_col % ch == 0

    xfb_bufs = 2 * cb_n * ncols + xfb_extra
    dc_ops = [nc.gpsimd.tensor_copy if e == "gpsimd"
              else nc.vector.tensor_copy if e == "vector"
              else nc.scalar.copy for e in dc_engines]

    with (
        tc.tile_pool(name="consts", bufs=1) as consts,
        tc.tile_pool(name="xf32", bufs=xf32_bufs) as xf32_pool,
        tc.tile_pool(name="xfb", bufs=xfb_bufs) as xfb_pool,
        tc.tile_pool(name="xft", bufs=xft_bufs) as xft_pool,
        tc.tile_pool(name="att", bufs=4) as att_pool,
        tc.tile_pool(name="attT", bufs=4) as attT_pool,
        tc.tile_pool(name="osb", bufs=osb_bufs) as osb_pool,
        tc.tile_pool(name="stat", bufs=4) as stat_pool,
        tc.tile_pool(name="eps", bufs=eps_bufs, space="PSUM") as eps_pool,
        tc.tile_pool(name="ptr", bufs=ptr_bufs, space="PSUM") as ptr_pool,
        tc.tile_pool(name="ptr2", bufs=ptr2_bufs, space="PSUM") as ptr2_pool,
        tc.tile_pool(name="pout", bufs=pout_bufs, space="PSUM") as pout_pool,
    ):
        identity = consts.tile([P, P], BF16, tag="identity")
        make_identity(nc, identity)
        identity_f = consts.tile([P, P], F32, tag="identity_f")
        make_identity(nc, identity_f)
        gamma_sb = consts.tile([P, 1], F32, tag="gamma")
        nc.sync.dma_start(gamma_sb, gamma_b)

        # -------- per-sample stage emitters (state dict per sample) --------
        def new_state(s):
            return {"s": s, "xfb": [[None] * ncols for _ in range(cb_n)],
                    "e_ps": None, "prev": None, "attT": None, "ginv": None,
                    "dc": 0}

        def emit_load_col(st, o, dve_only=False):
            s = st["s"]
            for cb in range(cb_n):
                t32 = xf32_pool.tile([P, w_col], F32, tag="xf32",
                                     name=f"x32_s{s}_c{cb}_o{o}")
                nc.sync.dma_start(
                    t32, x[s, cb * P:(cb + 1) * P, o * w_col:(o + 1) * w_col])
                tb = xfb_pool.tile([P, w_col], BF16, tag="xfb",
                                   name=f"xfb_s{s}_c{cb}_o{o}")
                if dve_only or o == ncols - 1:
                    # keep the copy off Pool: its 1.5us Q7 copy is the long
                    # pole feeding the first transpose of the column
                    nc.vector.tensor_copy(tb, t32)
                else:
                    dc_ops[st["dc"] % len(dc_ops)](tb, t32)
                st["dc"] += 1
                st["xfb"][cb][o] = tb

        def emit_tr(st, g):
            # four n-tiles (t = 4*g .. 4*g+3) transposed into one full-bank
            # PSUM tile and evicted with a single ScalarE copy
            s = st["s"]
            ptr = ptr_pool.tile([P, 4 * C], BF16, tag="ptr", name=f"ptr_s{s}_g{g}")
            for i in range(4):
                t = 4 * g + i
                o, lc = divmod(t * P, w_col)
                for cb in range(cb_n):
                    nc.tensor.transpose(
                        ptr[:, i * C + cb * P:i * C + (cb + 1) * P],
                        st["xfb"][cb][o][:, lc:lc + P], identity)
            xft = xft_pool.tile([P, 4 * C], BF16, tag="xft", name=f"xft_s{s}_g{g}")
            nc.scalar.copy(xft, ptr)
            return xft

        def emit_mm(st, g, xft):
            # symmetric Gram: rows 0:128 in full, rows 128:256 only for
            # cols 128:256 (the lower-left block is e0[:, 128:256].T)
            for i in range(4):
                t = 4 * g + i
                nc.tensor.matmul(
                    st["e_ps"][0],
                    lhsT=xft[:, i * C:i * C + P],
                    rhs=xft[:, i * C:(i + 1) * C],
                    start=(t == 0), stop=(t == 4 * ngr - 1))
                nc.tensor.matmul(
                    st["e_ps"][1],
                    lhsT=xft[:, i * C + P:i * C + 2 * P],
                    rhs=xft[:, i * C + P:(i + 1) * C],
                    start=(t == 0), stop=(t == 4 * ngr - 1))

        def p1_group(st, g):
            # software-pipelined: transposes+eviction for group g are emitted
            # before the accumulating matmuls of group g-1, so the PE stream
            # never blocks on the ScalarE eviction
            if st["e_ps"] is None:
                s = st["s"]
                e = eps_pool.tile([P, 2 * C], F32, tag="eps", name=f"eps_s{s}")
                st["e_ps"] = [e[:, 0:C], e[:, C:C + P], e[:, C + P:2 * C]]
            xft = emit_tr(st, g)
            if st["prev"] is not None:
                emit_mm(st, *st["prev"])
            st["prev"] = (g, xft)

        def p1_flush(st):
            if st["prev"] is not None:
                emit_mm(st, *st["prev"])
                st["prev"] = None

        def emit_softmax(st):
            # E = exp(rowmin - energy) (same as the reference's max-shifted
            # softmax); denom = rowsum(E); attT tiles in bf16 for phase 2.
            # Rows 128:256 reassemble their 0:128 columns from e0's upper
            # right block via one SBUF bounce + PE transpose.
            s = st["s"]
            e0, e11, e10 = st["e_ps"]
            s01 = att_pool.tile([P, P], F32, tag="s01", name=f"s01_{s}")
            nc.scalar.copy(s01, e0[:, P:C])
            nc.tensor.transpose(e10, s01, identity_f)
            att = []
            ginv = []
            for mb in range(cb_n):
                a = att_pool.tile([P, C], F32, tag="att", name=f"att_s{s}_{mb}")
                den = stat_pool.tile([P, 1], F32, tag="den", name=f"den_s{s}_{mb}")
                m = stat_pool.tile([P, 1], F32, tag="m", name=f"m_s{s}_{mb}")
                if mb == 0:
                    nc.vector.tensor_reduce(
                        m, e0, axis=mybir.AxisListType.X, op=mybir.AluOpType.min)
                    nc.scalar.activation(
                        a, e0, mybir.ActivationFunctionType.Exp,
                        bias=m, scale=-1.0, accum_out=den)
                else:
                    ma = stat_pool.tile([P, 1], F32, tag="ma", name=f"ma_{s}")
                    mb_ = stat_pool.tile([P, 1], F32, tag="mb", name=f"mb_{s}")
                    nc.vector.tensor_reduce(
                        ma, e10, axis=mybir.AxisListType.X, op=mybir.AluOpType.min)
                    nc.vector.tensor_reduce(
                        mb_, e11, axis=mybir.AxisListType.X, op=mybir.AluOpType.min)
                    nc.vector.tensor_tensor(m, ma, mb_, mybir.AluOpType.min)
                    db = stat_pool.tile([P, 1], F32, tag="db", name=f"db_{s}")
                    nc.scalar.activation(
                        a[:, 0:P], e10, mybir.ActivationFunctionType.Exp,
                        bias=m, scale=-1.0, accum_out=den)
                    nc.scalar.activation(
                        a[:, P:C], e11, mybir.ActivationFunctionType.Exp,
                        bias=m, scale=-1.0, accum_out=db)
                    nc.vector.tensor_tensor(den, den, db, mybir.AluOpType.add)
                inv = stat_pool.tile([P, 1], F32, tag="inv", name=f"inv_s{s}_{mb}")
                nc.vector.reciprocal(inv, den)
                gi = stat_pool.tile([P, 1], F32, tag="gi", name=f"gi_s{s}_{mb}")
                nc.vector.tensor_tensor(gi, inv, gamma_sb, mybir.AluOpType.mult)
                att.append(a)
                ginv.append(gi)
            ptr2 = ptr2_pool.tile([P, 2 * C], F32, tag="ptr2",
                                  name=f"ptrT_s{s}")
            for jb in range(cb_n):
                for ib in range(cb_n):
                    nc.tensor.transpose(
                        ptr2[:, jb * C + ib * P:jb * C + (ib + 1) * P],
                        att[ib][:, jb * P:(jb + 1) * P], identity_f)
            aT = attT_pool.tile([P, 2 * C], BF16, tag="attT", name=f"attT_s{s}")
            nc.scalar.copy(aT, ptr2)
            st["attT"] = [aT[:, jb * C:(jb + 1) * C] for jb in range(cb_n)]
            st["ginv"] = ginv

        def p2_compute(st, o):
            # out = gamma/denom * (attT.T @ x_bf16) + x_bf16 for one
            # [128, w_col] column per channel block
            s = st["s"]
            osbs = []
            for cb in range(cb_n):
                osb = osb_pool.tile([P, w_col], F32, tag="osb",
                                    name=f"osb_s{s}_o{o}_{cb}")
                for h in range(hpc):
                    po = pout_pool.tile([P, ch], F32, tag="pout",
                                        name=f"po_s{s}_o{o}_{cb}_{h}")
                    for jb in range(cb_n):
                        nc.tensor.matmul(
                            po,
                            lhsT=st["attT"][jb][:, cb * P:(cb + 1) * P],
                            rhs=st["xfb"][jb][o][:, h * ch:(h + 1) * ch],
                            start=(jb == 0), stop=(jb == cb_n - 1))
                    nc.vector.scalar_tensor_tensor(
                        osb[:, h * ch:(h + 1) * ch], po, st["ginv"][cb],
                        st["xfb"][cb][o][:, h * ch:(h + 1) * ch],
                        op0=mybir.AluOpType.mult, op1=mybir.AluOpType.add)
                osbs.append(osb)
            return osbs

        def p2_store(st, o, osbs):
            # stores issue from the Activation HWDGE queue so a store that
            # waits on softmax never head-of-line blocks loads on SP's queue
            s = st["s"]
            for cb in range(cb_n):
                nc.sync.dma_start(
                    out[s, cb * P:(cb + 1) * P, o * w_col:(o + 1) * w_col],
                    osbs[cb])

        def p2_col(st, o):
            p2_store(st, o, p2_compute(st, o))

        # -------- schedule --------
        # Three scheduling devices keep the DMA resource saturated across
        # sample transitions:
        #   - a burst of the next sample's loads is enqueued before this
        #     sample's first store, so the round-robin HW DMA lanes always
        #     have runnable loads while stores wait on softmax;
        #   - the next sample's phase-1 PE work trails the loads by one
        #     column so it never queues ahead of ready work;
        #   - the last `hold_cols` store columns of sample s are withheld
        #     and fired during sample s+1's softmax latency, when the DMA
        #     queue would otherwise run dry.
        states = [new_state(s) for s in range(n_s)]
        st0 = states[0]
        for o in range(ncols):
            emit_load_col(st0, o, dve_only=True)
            for k in range(gpc):
                p1_group(st0, o * gpc + k)
        p1_flush(st0)
        # burst of sample-1 loads BEFORE softmax(s0): the downcasts land
        # ahead of the softmax's DVE reductions in DVE program order, so
        # they don't stall behind a reduce that waits on the last matmul
        if n_s > 1:
            # burst downcasts stay on DVE: Pool's ~1.5us per-copy rate would
            # starve the small f32 staging pool during the transition
            for o in range(burst_cols):
                emit_load_col(states[1], o, dve_only=True)
        emit_softmax(st0)
        held = None
        for s in range(n_s):
            st = states[s]
            nxt = states[s + 1] if s + 1 < n_s else None
            hold = min(hold_cols, ncols) if nxt is not None else 0
            if nxt is not None and s > 0:
                for o in range(burst_cols):
                    emit_load_col(nxt, o)
            if held is not None:
                # previous sample's deferred stores: ready immediately, they
                # bridge the DMA gap while this sample's softmax resolves
                for o, osbs in held:
                    p2_store(states[s - 1], o, osbs)
                held = None
            new_held = []
            for o in range(ncols):
                osbs = p2_compute(st, o)
                if nxt is not None and burst_cols + o < ncols:
                    emit_load_col(nxt, burst_cols + o)
                if o >= ncols - hold:
                    new_held.append((o, osbs))
                else:
                    p2_store(st, o, osbs)
                if nxt is not None and o > 0:
                    for k in range(gpc):
                        p1_group(nxt, (o - 1) * gpc + k)
            held = new_held or None
            if nxt is not None:
                for k in range(gpc):
                    p1_group(nxt, (ncols - 1) * gpc + k)
                p1_flush(nxt)
                emit_softmax(nxt)
        if held:
            for o, osbs in held:
                p2_store(states[n_s - 1], o, osbs)


def build_nc(n_s=B_PER_CORE, C=C_FULL, N=H_FULL * W_FULL, **kwargs):
    nc = bacc.Bacc("TRN2", target_bir_lowering=False, debug=False)
    x = nc.dram_tensor("x", [n_s, C, N], F32, kind="ExternalInput").ap()
    gamma_b = nc.dram_tensor("gamma_b", [P, 1], F32, kind="ExternalInput").ap()
    out = nc.dram_tensor("out", [n_s, C, N], F32, kind="ExternalOutput").ap()
    with tile.TileContext(nc) as tc:
        emit_cam(tc, x, gamma_b, out, n_s, C, N, **kwargs)
    nc.compile()
    return nc


_CACHE = threading.Lock()
_NC = None


def _get_nc():
    global _NC
    with _CACHE:
        if _NC is None:
            _NC = build_nc()
    return _NC


def run_spmd(x, gamma, **kwargs):
    """Shard inputs over 8 cores, run, gather. Returns (output, BassKernelResults)."""
    x = np.ascontiguousarray(np.asarray(x), dtype=np.float32)
    assert x.shape == (B_FULL, C_FULL, H_FULL, W_FULL), x.shape
    n = H_FULL * W_FULL
    xs = x.reshape(B_FULL, C_FULL, n)
    gb = np.full((P, 1), np.float32(np.asarray(gamma)), dtype=np.float32)
    in_maps = [
        {"x": xs[c * B_PER_CORE:(c + 1) * B_PER_CORE], "gamma_b": gb}
        for c in range(N_CORES)
    ]
    nc = _get_nc()
    res = run_bass_kernel_spmd(nc, in_maps, core_ids=list(range(N_CORES)), **kwargs)
    outs = np.stack([res.results[c]["out"] for c in range(N_CORES)])
    full = outs.reshape(B_FULL, C_FULL, H_FULL, W_FULL).astype(np.float32, copy=False)
    return full, res


def kernel(x, gamma):
    out, _ = run_spmd(x, gamma)
    return out


# revision 20
# speedup vs baseline: 1.0277x; 1.0277x over previous
"""CAM (channel attention module) Trainium2 Bass kernel.

Reference computation (per sample, x: [C, N] with N = H*W):
    energy    = x @ x.T                      # [C, C] Gram matrix
    att       = softmax(rowmax(energy) - energy, axis=-1)
              = softmax(-energy, axis=-1)    # identical after max-shift
    out       = att @ x                      # [C, N]
    result    = gamma * out + x

Sharding: data-parallel over batch, B=16 -> 2 samples per core on 8 cores.

The kernel is HBM-bound: 16 MiB in + 16 MiB out per sample, 64 MiB per
core against the ~360 B/ns DMA_ENGINES roofline = 186.4 us.  The design
goal is to keep the DMA resource 100% occupied:

  - x is loaded in [128, 1024] f32 tiles (4 KiB descriptors) through a
    small staging pool and immediately downcast to a RESIDENT bf16 copy
    (8 MiB/sample instead of 16 MiB f32), alternating Pool/DVE so the
    downcast rate beats the load rate.  Halved residency means the next
    sample's loads are never buffer-starved: the DMA queue always has
    either loads or stores available.
  - Phase 1 transposes run on bf16 (1 cyc/row vs 2 for f32) so the PE
    keeps pace with the load stream; two 128-col transposes batch into
    one [128, 512] PSUM tile and a single ScalarE eviction (the ACT
    fixed overhead is ~185 ns/instruction).
  - The energy Gram matrix accumulates in PSUM f32 from bf16 operands;
    softmax is the min-shift rewrite of the reference's max-shifted
    form, with the 1/denom and gamma folded into one per-row scalar.
  - Phase 2 computes gamma/denom * (attT.T @ x_bf16) + x_bf16 per
    [128, 512] PSUM chunk, pairing chunks into [128, 1024] stores.
    The +x residual uses the bf16 copy (rel-err ~1e-3, far inside the
    2e-2 gate); with the reference's gamma == 0 the attention term
    vanishes entirely and the output is bf16-rounded x.
  - Sample s+1's loads are emitted ahead of sample s's stores each
    column so the 8 round-robin HW DMA lanes never head-of-line block
    on a store that waits for softmax.
"""

import threading

import numpy as np

import concourse.bass as bass
import concourse.mybir as mybir
import concourse.tile as tile
from concourse import bacc
from concourse.bass_utils import run_bass_kernel_spmd
from concourse.masks import make_identity

P = 128
F32 = mybir.dt.float32
BF16 = mybir.dt.bfloat16

# Full-problem shapes (hardcoded per harness contract).
B_FULL = 16
C_FULL = 256
H_FULL = W_FULL = 128
N_CORES = 8
B_PER_CORE = B_FULL // N_CORES  # 2


def emit_cam(tc, x, gamma_b, out, n_s, C, N, w_col=1024, ch=512,
             xf32_bufs=8, xfb_extra=0, xft_bufs=6, osb_bufs=6,
             ptr_bufs=3, pout_bufs=2, eps_bufs=2, ptr2_bufs=1,
             burst_cols=8, hold_cols=2,
             dc_engines=("gpsimd", "vector")):
    """Emit the per-core CAM kernel.

    x:       DRAM [n_s, C, N] f32
    gamma_b: DRAM [128, 1] f32 (gamma broadcast to all partitions on host)
    out:     DRAM [n_s, C, N] f32
    """
    nc = tc.nc
    cb_n = C // P                 # channel blocks (2)
    ncols = N // w_col            # load/store columns per sample (16)
    tpc = w_col // P              # transpose tiles per column (8)
    gpc = tpc // 4                # transpose groups (of 4 tiles) per column (2)
    ngr = N // (4 * P)            # transpose groups per sample (32)
    hpc = w_col // ch             # phase-2 psum chunks per column (2)
    assert C == 2 * P and w_col % (4 * P) == 0 and w_col % ch == 0

    xfb_bufs = 2 * cb_n * ncols + xfb_extra
    dc_ops = [nc.gpsimd.tensor_copy if e == "gpsimd"
              else nc.vector.tensor_copy if e == "vector"
              else nc.scalar.copy for e in dc_engines]

    with (
        tc.tile_pool(name="consts", bufs=1) as consts,
        tc.tile_pool(name="xf32", bufs=xf32_bufs) as xf32_pool,
        tc.tile_pool(name="xfb", bufs=xfb_bufs) as xfb_pool,
        tc.tile_pool(name="xft", bufs=xft_bufs) as xft_pool,
        tc.tile_pool(name="att", bufs=4) as att_pool,
        tc.tile_pool(name="attT", bufs=4) as attT_pool,
        tc.tile_pool(name="osb", bufs=osb_bufs) as osb_pool,
        tc.tile_pool(name="stat", bufs=4) as stat_pool,
        tc.tile_pool(name="eps", bufs=eps_bufs, space="PSUM") as eps_pool,
        tc.tile_pool(name="ptr", bufs=ptr_bufs, space="PSUM") as ptr_pool,
        tc.tile_pool(name="ptr2", bufs=ptr2_bufs, space="PSUM") as ptr2_pool,
        tc.tile_pool(name="pout", bufs=pout_bufs, space="PSUM") as pout_pool,
    ):
        identity = consts.tile([P, P], BF16, tag="identity")
        identity_f = consts.tile([P, P], F32, tag="identity_f")
        gamma_sb = consts.tile([P, 1], F32, tag="gamma")

        def emit_consts():
            make_identity(nc, identity)
            make_identity(nc, identity_f)
            nc.sync.dma_start(gamma_sb, gamma_b)

        # -------- per-sample stage emitters (state dict per sample) --------
        def new_state(s):
            return {"s": s, "xfb": [[None] * ncols for _ in range(cb_n)],
                    "e_ps": None, "prev": None, "attT": None, "ginv": None,
                    "dc": 0}

        def emit_load_col(st, o, dve_only=False):
            s = st["s"]
            for cb in range(cb_n):
                t32 = xf32_pool.tile([P, w_col], F32, tag="xf32",
                                     name=f"x32_s{s}_c{cb}_o{o}")
                nc.sync.dma_start(
                    t32, x[s, cb * P:(cb + 1) * P, o * w_col:(o + 1) * w_col])
                tb = xfb_pool.tile([P, w_col], BF16, tag="xfb",
                                   name=f"xfb_s{s}_c{cb}_o{o}")
                if dve_only or o == ncols - 1:
                    # keep the copy off Pool: its 1.5us Q7 copy is the long
                    # pole feeding the first transpose of the column
                    nc.vector.tensor_copy(tb, t32)
                else:
                    dc_ops[st["dc"] % len(dc_ops)](tb, t32)
                st["dc"] += 1
                st["xfb"][cb][o] = tb

        def emit_tr(st, g):
            # four n-tiles (t = 4*g .. 4*g+3) transposed into one full-bank
            # PSUM tile and evicted with a single ScalarE copy
            s = st["s"]
            ptr = ptr_pool.tile([P, 4 * C], BF16, tag="ptr", name=f"ptr_s{s}_g{g}")
            for i in range(4):
                t = 4 * g + i
                o, lc = divmod(t * P, w_col)
                for cb in range(cb_n):
                    nc.tensor.transpose(
                        ptr[:, i * C + cb * P:i * C + (cb + 1) * P],
                        st["xfb"][cb][o][:, lc:lc + P], identity)
            xft = xft_pool.tile([P, 4 * C], BF16, tag="xft", name=f"xft_s{s}_g{g}")
            nc.scalar.copy(xft, ptr)
            return xft

        def emit_mm(st, g, xft):
            # symmetric Gram: rows 0:128 in full, rows 128:256 only for
            # cols 128:256 (the lower-left block is e0[:, 128:256].T)
            for i in range(4):
                t = 4 * g + i
                nc.tensor.matmul(
                    st["e_ps"][0],
                    lhsT=xft[:, i * C:i * C + P],
                    rhs=xft[:, i * C:(i + 1) * C],
                    start=(t == 0), stop=(t == 4 * ngr - 1))
                nc.tensor.matmul(
                    st["e_ps"][1],
                    lhsT=xft[:, i * C + P:i * C + 2 * P],
                    rhs=xft[:, i * C + P:(i + 1) * C],
                    start=(t == 0), stop=(t == 4 * ngr - 1))

        def p1_group(st, g):
            # software-pipelined: transposes+eviction for group g are emitted
            # before the accumulating matmuls of group g-1, so the PE stream
            # never blocks on the ScalarE eviction
            if st["e_ps"] is None:
                s = st["s"]
                e = eps_pool.tile([P, 2 * C], F32, tag="eps", name=f"eps_s{s}")
                st["e_ps"] = [e[:, 0:C], e[:, C:C + P], e[:, C + P:2 * C]]
            xft = emit_tr(st, g)
            if st["prev"] is not None:
                emit_mm(st, *st["prev"])
            st["prev"] = (g, xft)

        def p1_flush(st):
            if st["prev"] is not None:
                emit_mm(st, *st["prev"])
                st["prev"] = None

        def emit_softmax(st):
            # E = exp(rowmin - energy) (same as the reference's max-shifted
            # softmax); denom = rowsum(E); attT tiles in bf16 for phase 2.
            # Rows 128:256 reassemble their 0:128 columns from e0's upper
            # right block via one SBUF bounce + PE transpose.
            s = st["s"]
            e0, e11, e10 = st["e_ps"]
            s01 = att_pool.tile([P, P], F32, tag="s01", name=f"s01_{s}")
            nc.scalar.copy(s01, e0[:, P:C])
            nc.tensor.transpose(e10, s01, identity_f)
            att = []
            ginv = []
            for mb in range(cb_n):
                a = att_pool.tile([P, C], F32, tag="att", name=f"att_s{s}_{mb}")
                den = stat_pool.tile([P, 1], F32, tag="den", name=f"den_s{s}_{mb}")
                m = stat_pool.tile([P, 1], F32, tag="m", name=f"m_s{s}_{mb}")
                if mb == 0:
                    nc.vector.tensor_reduce(
                        m, e0, axis=mybir.AxisListType.X, op=mybir.AluOpType.min)
                    nc.scalar.activation(
                        a, e0, mybir.ActivationFunctionType.Exp,
                        bias=m, scale=-1.0, accum_out=den)
                else:
                    ma = stat_pool.tile([P, 1], F32, tag="ma", name=f"ma_{s}")
                    mb_ = stat_pool.tile([P, 1], F32, tag="mb", name=f"mb_{s}")
                    nc.vector.tensor_reduce(
                        ma, e10, axis=mybir.AxisListType.X, op=mybir.AluOpType.min)
                    nc.vector.tensor_reduce(
                        mb_, e11, axis=mybir.AxisListType.X, op=mybir.AluOpType.min)
                    nc.vector.tensor_tensor(m, ma, mb_, mybir.AluOpType.min)
                    db = stat_pool.tile([P, 1], F32, tag="db", name=f"db_{s}")
                    nc.scalar.activation(
                        a[:, 0:P], e10, mybir.ActivationFunctionType.Exp,
                        bias=m, scale=-1.0, accum_out=den)
                    nc.scalar.activation(
                        a[:, P:C], e11, mybir.ActivationFunctionType.Exp,
                        bias=m, scale=-1.0, accum_out=db)
                    nc.vector.tensor_tensor(den, den, db, mybir.AluOpType.add)
                inv = stat_pool.tile([P, 1], F32, tag="inv", name=f"inv_s{s}_{mb}")
                nc.vector.reciprocal(inv, den)
                gi = stat_pool.tile([P, 1], F32, tag="gi", name=f"gi_s{s}_{mb}")
                nc.vector.tensor_tensor(gi, inv, gamma_sb, mybir.AluOpType.mult)
                att.append(a)
                ginv.append(gi)
            ptr2 = ptr2_pool.tile([P, 2 * C], F32, tag="ptr2",
                                  name=f"ptrT_s{s}")
            for jb in range(cb_n):
                for ib in range(cb_n):
                    nc.tensor.transpose(
                        ptr2[:, jb * C + ib * P:jb * C + (ib + 1) * P],
                        att[ib][:, jb * P:(jb + 1) * P], identity_f)
            aT = attT_pool.tile([P, 2 * C], BF16, tag="attT", name=f"attT_s{s}")
            nc.scalar.copy(aT, ptr2)
            st["attT"] = [aT[:, jb * C:(jb + 1) * C] for jb in range(cb_n)]
            st["ginv"] = ginv

        def p2_compute(st, o):
            # out = gamma/denom * (attT.T @ x_bf16) + x_bf16 for one
            # [128, w_col] column per channel block
            s = st["s"]
            osbs = []
            for cb in range(cb_n):
                osb = osb_pool.tile([P, w_col], F32, tag="osb",
                                    name=f"osb_s{s}_o{o}_{cb}")
                for h in range(hpc):
                    po = pout_pool.tile([P, ch], F32, tag="pout",
                                        name=f"po_s{s}_o{o}_{cb}_{h}")
                    for jb in range(cb_n):
                        nc.tensor.matmul(
                            po,
                            lhsT=st["attT"][jb][:, cb * P:(cb + 1) * P],
                            rhs=st["xfb"][jb][o][:, h * ch:(h + 1) * ch],
                            start=(jb == 0), stop=(jb == cb_n - 1))
                    nc.vector.scalar_tensor_tensor(
                        osb[:, h * ch:(h + 1) * ch], po, st["ginv"][cb],
                        st["xfb"][cb][o][:, h * ch:(h + 1) * ch],
                        op0=mybir.AluOpType.mult, op1=mybir.AluOpType.add)
                osbs.append(osb)
            return osbs

        def p2_store(st, o, osbs):
            # stores issue from the Activation HWDGE queue so a store that
            # waits on softmax never head-of-line blocks loads on SP's queue
            s = st["s"]
            for cb in range(cb_n):
                nc.sync.dma_start(
                    out[s, cb * P:(cb + 1) * P, o * w_col:(o + 1) * w_col],
                    osbs[cb])

        def p2_col(st, o):
            p2_store(st, o, p2_compute(st, o))

        # -------- schedule --------
        # Three scheduling devices keep the DMA resource saturated across
        # sample transitions:
        #   - a burst of the next sample's loads is enqueued before this
        #     sample's first store, so the round-robin HW DMA lanes always
        #     have runnable loads while stores wait on softmax;
        #   - the next sample's phase-1 PE work trails the loads by one
        #     column so it never queues ahead of ready work;
        #   - the last `hold_cols` store columns of sample s are withheld
        #     and fired during sample s+1's softmax latency, when the DMA
        #     queue would otherwise run dry.
        states = [new_state(s) for s in range(n_s)]
        st0 = states[0]
        for o in range(ncols):
            emit_load_col(st0, o, dve_only=True)
            if o == 0:
                # consts after the first loads so the SP DMA queue's head is
                # real work, not the 1-element gamma broadcast
                emit_consts()
            for k in range(gpc):
                p1_group(st0, o * gpc + k)
        p1_flush(st0)
        # burst of sample-1 loads BEFORE softmax(s0): the downcasts land
        # ahead of the softmax's DVE reductions in DVE program order, so
        # they don't stall behind a reduce that waits on the last matmul
        if n_s > 1:
            # burst downcasts stay on DVE: Pool's ~1.5us per-copy rate would
            # starve the small f32 staging pool during the transition
            for o in range(burst_cols):
                emit_load_col(states[1], o, dve_only=True)
        emit_softmax(st0)
        held = None
        for s in range(n_s):
            st = states[s]
            nxt = states[s + 1] if s + 1 < n_s else None
            hold = min(hold_cols, ncols) if nxt is not None else 0
            if nxt is not None and s > 0:
                for o in range(burst_cols):
                    emit_load_col(nxt, o)
            if held is not None:
                # previous sample's deferred stores: ready immediately, they
                # bridge the DMA gap while this sample's softmax resolves
                for o, osbs in held:
                    p2_store(states[s - 1], o, osbs)
                held = None
            new_held = []
            for o in range(ncols):
                osbs = p2_compute(st, o)
                if nxt is not None and burst_cols + o < ncols:
                    emit_load_col(nxt, burst_cols + o)
                if o >= ncols - hold:
                    new_held.append((o, osbs))
                else:
                    p2_store(st, o, osbs)
                if nxt is not None and o > 0:
                    for k in range(gpc):
                        p1_group(nxt, (o - 1) * gpc + k)
            held = new_held or None
            if nxt is not None:
                for k in range(gpc):
                    p1_group(nxt, (ncols - 1) * gpc + k)
                p1_flush(nxt)
                emit_softmax(nxt)
        if held:
            for o, osbs in held:
                p2_store(states[n_s - 1], o, osbs)


def build_nc(n_s=B_PER_CORE, C=C_FULL, N=H_FULL * W_FULL, **kwargs):
    nc = bacc.Bacc("TRN2", target_bir_lowering=False, debug=False)
    x = nc.dram_tensor("x", [n_s, C, N], F32, kind="ExternalInput").ap()
    gamma_b = nc.dram_tensor("gamma_b", [P, 1], F32, kind="ExternalInput").ap()
    out = nc.dram_tensor("out", [n_s, C, N], F32, kind="ExternalOutput").ap()
    with tile.TileContext(nc) as tc:
        emit_cam(tc, x, gamma_b, out, n_s, C, N, **kwargs)
    nc.compile()
    return nc


_CACHE = threading.Lock()
_NC = None


def _get_nc():
    global _NC
    with _CACHE:
        if _NC is None:
            _NC = build_nc()
    return _NC


def run_spmd(x, gamma, **kwargs):
    """Shard inputs over 8 cores, run, gather. Returns (output, BassKernelResults)."""
    x = np.ascontiguousarray(np.asarray(x), dtype=np.float32)
    assert x.shape == (B_FULL, C_FULL, H_FULL, W_FULL), x.shape
    n = H_FULL * W_FULL
    xs = x.reshape(B_FULL, C_FULL, n)
    gb = np.full((P, 1), np.float32(np.asarray(gamma)), dtype=np.float32)
    in_maps = [
        {"x": xs[c * B_PER_CORE:(c + 1) * B_PER_CORE], "gamma_b": gb}
        for c in range(N_CORES)
    ]
    nc = _get_nc()
    res = run_bass_kernel_spmd(nc, in_maps, core_ids=list(range(N_CORES)), **kwargs)
    outs = np.stack([res.results[c]["out"] for c in range(N_CORES)])
    full = outs.reshape(B_FULL, C_FULL, H_FULL, W_FULL).astype(np.float32, copy=False)
    return full, res


def kernel(x, gamma):
    out, _ = run_spmd(x, gamma)
    return out


# revision 22
# speedup vs baseline: 1.1065x; 1.0767x over previous
"""CAM (channel attention module) Trainium2 Bass kernel.

Reference computation (per sample, x: [C, N] with N = H*W):
    energy    = x @ x.T                      # [C, C] Gram matrix
    att       = softmax(rowmax(energy) - energy, axis=-1)
              = softmax(-energy, axis=-1)    # identical after max-shift
    out       = att @ x                      # [C, N]
    result    = gamma * out + x

Sharding: data-parallel over batch, B=16 -> 2 samples per core on 8 cores.

The kernel is HBM-bound: 16 MiB in + 16 MiB out per sample, 64 MiB per
core against the ~360 B/ns DMA_ENGINES roofline = 186.4 us.  The design
goal is to keep the DMA resource 100% occupied:

  - x is loaded in [128, 1024] f32 tiles (4 KiB descriptors) through a
    small staging pool and immediately downcast to a RESIDENT bf16 copy
    (8 MiB/sample instead of 16 MiB f32), alternating Pool/DVE so the
    downcast rate beats the load rate.  Halved residency means the next
    sample's loads are never buffer-starved: the DMA queue always has
    either loads or stores available.
  - Phase 1 transposes run on bf16 (1 cyc/row vs 2 for f32) so the PE
    keeps pace with the load stream; two 128-col transposes batch into
    one [128, 512] PSUM tile and a single ScalarE eviction (the ACT
    fixed overhead is ~185 ns/instruction).
  - The energy Gram matrix accumulates in PSUM f32 from bf16 operands;
    softmax is the min-shift rewrite of the reference's max-shifted
    form, with the 1/denom and gamma folded into one per-row scalar.
  - Phase 2 computes gamma/denom * (attT.T @ x_bf16) + x_bf16 per
    [128, 512] PSUM chunk, pairing chunks into [128, 1024] stores.
    The +x residual uses the bf16 copy (rel-err ~1e-3, far inside the
    2e-2 gate); with the reference's gamma == 0 the attention term
    vanishes entirely and the output is bf16-rounded x.
  - Sample s+1's loads are emitted ahead of sample s's stores each
    column so the 8 round-robin HW DMA lanes never head-of-line block
    on a store that waits for softmax.
"""

import threading

import numpy as np

import concourse.bass as bass
import concourse.mybir as mybir
import concourse.tile as tile
from concourse import bacc
from concourse.bass_utils import run_bass_kernel_spmd
from concourse.masks import make_identity

P = 128
F32 = mybir.dt.float32
BF16 = mybir.dt.bfloat16

# Full-problem shapes (hardcoded per harness contract).
B_FULL = 16
C_FULL = 256
H_FULL = W_FULL = 128
N_CORES = 8
B_PER_CORE = B_FULL // N_CORES  # 2


def emit_cam(tc, x, gamma_b, out, n_s, C, N, w_col=1024, ch=512,
             xf32_bufs=8, xfb_extra=0, xft_bufs=6, osb_bufs=6,
             ptr_bufs=3, pout_bufs=2, eps_bufs=2, ptr2_bufs=1,
             burst_cols=8, hold_cols=2, burst_dc="dve",
             dc_engines=("gpsimd", "vector")):
    """Emit the per-core CAM kernel.

    x:       DRAM [n_s, C, N] f32
    gamma_b: DRAM [128, 1] f32 (gamma broadcast to all partitions on host)
    out:     DRAM [n_s, C, N] f32
    """
    nc = tc.nc
    cb_n = C // P                 # channel blocks (2)
    ncols = N // w_col            # load/store columns per sample (16)
    tpc = w_col // P              # transpose tiles per column (8)
    gpc = tpc // 4                # transpose groups (of 4 tiles) per column (2)
    ngr = N // (4 * P)            # transpose groups per sample (32)
    hpc = w_col // ch             # phase-2 psum chunks per column (2)
    assert C == 2 * P and w_col % (4 * P) == 0 and w_col % ch == 0

    xfb_bufs = 2 * cb_n * ncols + xfb_extra
    dc_ops = [nc.gpsimd.tensor_copy if e == "gpsimd"
              else nc.vector.tensor_copy if e == "vector"
              else nc.scalar.copy for e in dc_engines]

    with (
        tc.tile_pool(name="consts", bufs=1) as consts,
        tc.tile_pool(name="xf32", bufs=xf32_bufs) as xf32_pool,
        tc.tile_pool(name="xfb", bufs=xfb_bufs) as xfb_pool,
        tc.tile_pool(name="xft", bufs=xft_bufs) as xft_pool,
        tc.tile_pool(name="att", bufs=4) as att_pool,
        tc.tile_pool(name="attT", bufs=4) as attT_pool,
        tc.tile_pool(name="osb", bufs=osb_bufs) as osb_pool,
        tc.tile_pool(name="stat", bufs=4) as stat_pool,
        tc.tile_pool(name="eps", bufs=eps_bufs, space="PSUM") as eps_pool,
        tc.tile_pool(name="ptr", bufs=ptr_bufs, space="PSUM") as ptr_pool,
        tc.tile_pool(name="ptr2", bufs=ptr2_bufs, space="PSUM") as ptr2_pool,
        tc.tile_pool(name="pout", bufs=pout_bufs, space="PSUM") as pout_pool,
    ):
        identity = consts.tile([P, P], BF16, tag="identity")
        identity_f = consts.tile([P, P], F32, tag="identity_f")
        gamma_sb = consts.tile([P, 1], F32, tag="gamma")

        def emit_consts():
            make_identity(nc, identity)
            make_identity(nc, identity_f)
            nc.sync.dma_start(gamma_sb, gamma_b)

        # -------- per-sample stage emitters (state dict per sample) --------
        def new_state(s):
            return {"s": s, "xfb": [[None] * ncols for _ in range(cb_n)],
                    "e_ps": None, "prev": None, "attT": None, "ginv": None,
                    "dc": 0}

        def emit_load_col(st, o, dc="alt"):
            s = st["s"]
            for cb in range(cb_n):
                t32 = xf32_pool.tile([P, w_col], F32, tag="xf32",
                                     name=f"x32_s{s}_c{cb}_o{o}")
                nc.sync.dma_start(
                    t32, x[s, cb * P:(cb + 1) * P, o * w_col:(o + 1) * w_col])
                tb = xfb_pool.tile([P, w_col], BF16, tag="xfb",
                                   name=f"xfb_s{s}_c{cb}_o{o}")
                if dc == "dve" or o == ncols - 1:
                    # keep the copy off Pool: its 1.5us Q7 copy is the long
                    # pole feeding the first transpose of the column
                    nc.vector.tensor_copy(tb, t32)
                elif dc == "pool":
                    nc.gpsimd.tensor_copy(tb, t32)
                else:
                    dc_ops[st["dc"] % len(dc_ops)](tb, t32)
                st["dc"] += 1
                st["xfb"][cb][o] = tb

        def emit_tr(st, g):
            # four n-tiles (t = 4*g .. 4*g+3) transposed into one full-bank
            # PSUM tile and evicted with a single ScalarE copy
            s = st["s"]
            ptr = ptr_pool.tile([P, 4 * C], BF16, tag="ptr", name=f"ptr_s{s}_g{g}")
            for i in range(4):
                t = 4 * g + i
                o, lc = divmod(t * P, w_col)
                for cb in range(cb_n):
                    nc.tensor.transpose(
                        ptr[:, i * C + cb * P:i * C + (cb + 1) * P],
                        st["xfb"][cb][o][:, lc:lc + P], identity)
            xft = xft_pool.tile([P, 4 * C], BF16, tag="xft", name=f"xft_s{s}_g{g}")
            nc.scalar.copy(xft, ptr)
            return xft

        def emit_mm(st, g, xft):
            # symmetric Gram: rows 0:128 in full, rows 128:256 only for
            # cols 128:256 (the lower-left block is e0[:, 128:256].T)
            for i in range(4):
                t = 4 * g + i
                nc.tensor.matmul(
                    st["e_ps"][0],
                    lhsT=xft[:, i * C:i * C + P],
                    rhs=xft[:, i * C:(i + 1) * C],
                    start=(t == 0), stop=(t == 4 * ngr - 1))
                nc.tensor.matmul(
                    st["e_ps"][1],
                    lhsT=xft[:, i * C + P:i * C + 2 * P],
                    rhs=xft[:, i * C + P:(i + 1) * C],
                    start=(t == 0), stop=(t == 4 * ngr - 1))

        def p1_group(st, g):
            # software-pipelined: transposes+eviction for group g are emitted
            # before the accumulating matmuls of group g-1, so the PE stream
            # never blocks on the ScalarE eviction
            if st["e_ps"] is None:
                s = st["s"]
                e = eps_pool.tile([P, 2 * C], F32, tag="eps", name=f"eps_s{s}")
                st["e_ps"] = [e[:, 0:C], e[:, C:C + P], e[:, C + P:2 * C]]
            xft = emit_tr(st, g)
            if st["prev"] is not None:
                emit_mm(st, *st["prev"])
            st["prev"] = (g, xft)

        def p1_flush(st):
            if st["prev"] is not None:
                emit_mm(st, *st["prev"])
                st["prev"] = None

        def emit_softmax(st):
            # E = exp(rowmin - energy) (same as the reference's max-shifted
            # softmax); denom = rowsum(E); attT tiles in bf16 for phase 2.
            # Rows 128:256 reassemble their 0:128 columns from e0's upper
            # right block via one SBUF bounce + PE transpose.
            s = st["s"]
            e0, e11, e10 = st["e_ps"]
            s01 = att_pool.tile([P, P], F32, tag="s01", name=f"s01_{s}")
            nc.scalar.copy(s01, e0[:, P:C])
            nc.tensor.transpose(e10, s01, identity_f)
            att = []
            ginv = []
            for mb in range(cb_n):
                a = att_pool.tile([P, C], F32, tag="att", name=f"att_s{s}_{mb}")
                den = stat_pool.tile([P, 1], F32, tag="den", name=f"den_s{s}_{mb}")
                m = stat_pool.tile([P, 1], F32, tag="m", name=f"m_s{s}_{mb}")
                if mb == 0:
                    nc.vector.tensor_reduce(
                        m, e0, axis=mybir.AxisListType.X, op=mybir.AluOpType.min)
                    nc.scalar.activation(
                        a, e0, mybir.ActivationFunctionType.Exp,
                        bias=m, scale=-1.0, accum_out=den)
                else:
                    ma = stat_pool.tile([P, 1], F32, tag="ma", name=f"ma_{s}")
                    mb_ = stat_pool.tile([P, 1], F32, tag="mb", name=f"mb_{s}")
                    nc.vector.tensor_reduce(
                        ma, e10, axis=mybir.AxisListType.X, op=mybir.AluOpType.min)
                    nc.vector.tensor_reduce(
                        mb_, e11, axis=mybir.AxisListType.X, op=mybir.AluOpType.min)
                    nc.vector.tensor_tensor(m, ma, mb_, mybir.AluOpType.min)
                    db = stat_pool.tile([P, 1], F32, tag="db", name=f"db_{s}")
                    nc.scalar.activation(
                        a[:, 0:P], e10, mybir.ActivationFunctionType.Exp,
                        bias=m, scale=-1.0, accum_out=den)
                    nc.scalar.activation(
                        a[:, P:C], e11, mybir.ActivationFunctionType.Exp,
                        bias=m, scale=-1.0, accum_out=db)
                    nc.vector.tensor_tensor(den, den, db, mybir.AluOpType.add)
                inv = stat_pool.tile([P, 1], F32, tag="inv", name=f"inv_s{s}_{mb}")
                nc.vector.reciprocal(inv, den)
                gi = stat_pool.tile([P, 1], F32, tag="gi", name=f"gi_s{s}_{mb}")
                nc.vector.tensor_tensor(gi, inv, gamma_sb, mybir.AluOpType.mult)
                att.append(a)
                ginv.append(gi)
            ptr2 = ptr2_pool.tile([P, 2 * C], F32, tag="ptr2",
                                  name=f"ptrT_s{s}")
            for jb in range(cb_n):
                for ib in range(cb_n):
                    nc.tensor.transpose(
                        ptr2[:, jb * C + ib * P:jb * C + (ib + 1) * P],
                        att[ib][:, jb * P:(jb + 1) * P], identity_f)
            aT = attT_pool.tile([P, 2 * C], BF16, tag="attT", name=f"attT_s{s}")
            nc.scalar.copy(aT, ptr2)
            st["attT"] = [aT[:, jb * C:(jb + 1) * C] for jb in range(cb_n)]
            st["ginv"] = ginv

        def p2_compute(st, o):
            # out = gamma/denom * (attT.T @ x_bf16) + x_bf16 for one
            # [128, w_col] column per channel block
            s = st["s"]
            osbs = []
            for cb in range(cb_n):
                osb = osb_pool.tile([P, w_col], F32, tag="osb",
                                    name=f"osb_s{s}_o{o}_{cb}")
                for h in range(hpc):
                    po = pout_pool.tile([P, ch], F32, tag="pout",
                                        name=f"po_s{s}_o{o}_{cb}_{h}")
                    for jb in range(cb_n):
                        nc.tensor.matmul(
                            po,
                            lhsT=st["attT"][jb][:, cb * P:(cb + 1) * P],
                            rhs=st["xfb"][jb][o][:, h * ch:(h + 1) * ch],
                            start=(jb == 0), stop=(jb == cb_n - 1))
                    nc.vector.scalar_tensor_tensor(
                        osb[:, h * ch:(h + 1) * ch], po, st["ginv"][cb],
                        st["xfb"][cb][o][:, h * ch:(h + 1) * ch],
                        op0=mybir.AluOpType.mult, op1=mybir.AluOpType.add)
                osbs.append(osb)
            return osbs

        def p2_store(st, o, osbs):
            # stores issue from the Activation HWDGE queue so a store that
            # waits on softmax never head-of-line blocks loads on SP's queue
            s = st["s"]
            for cb in range(cb_n):
                nc.sync.dma_start(
                    out[s, cb * P:(cb + 1) * P, o * w_col:(o + 1) * w_col],
                    osbs[cb])

        def p2_col(st, o):
            p2_store(st, o, p2_compute(st, o))

        # -------- schedule --------
        # Three scheduling devices keep the DMA resource saturated across
        # sample transitions:
        #   - a burst of the next sample's loads is enqueued before this
        #     sample's first store, so the round-robin HW DMA lanes always
        #     have runnable loads while stores wait on softmax;
        #   - the next sample's phase-1 PE work trails the loads by one
        #     column so it never queues ahead of ready work;
        #   - the last `hold_cols` store columns of sample s are withheld
        #     and fired during sample s+1's softmax latency, when the DMA
        #     queue would otherwise run dry.
        states = [new_state(s) for s in range(n_s)]
        st0 = states[0]
        for o in range(ncols):
            emit_load_col(st0, o, dc="dve")
            if o == 0:
                # consts after the first loads so the SP DMA queue's head is
                # real work, not the 1-element gamma broadcast
                emit_consts()
            for k in range(gpc):
                p1_group(st0, o * gpc + k)
        p1_flush(st0)
        # burst of sample-1 loads BEFORE softmax(s0): the downcasts land
        # ahead of the softmax's DVE reductions in DVE program order, so
        # they don't stall behind a reduce that waits on the last matmul
        if n_s > 1:
            # burst downcasts on DVE: Pool's ~1.5us per-copy rate would
            # starve the small f32 staging pool during the transition
            for o in range(burst_cols):
                emit_load_col(states[1], o, dc=burst_dc)
        emit_softmax(st0)
        held = None
        for s in range(n_s):
            st = states[s]
            nxt = states[s + 1] if s + 1 < n_s else None
            hold = min(hold_cols, ncols) if nxt is not None else 0
            if nxt is not None and s > 0:
                for o in range(burst_cols):
                    emit_load_col(nxt, o)
            if held is not None:
                # previous sample's deferred stores: ready immediately, they
                # bridge the DMA gap while this sample's softmax resolves
                for o, osbs in held:
                    p2_store(states[s - 1], o, osbs)
                held = None
            new_held = []
            for o in range(ncols):
                osbs = p2_compute(st, o)
                if nxt is not None and burst_cols + o < ncols:
                    emit_load_col(nxt, burst_cols + o)
                if o >= ncols - hold:
                    new_held.append((o, osbs))
                else:
                    p2_store(st, o, osbs)
                if nxt is not None and o > 0:
                    for k in range(gpc):
                        p1_group(nxt, (o - 1) * gpc + k)
            held = new_held or None
            if nxt is not None:
                for k in range(gpc):
                    p1_group(nxt, (ncols - 1) * gpc + k)
                p1_flush(nxt)
                emit_softmax(nxt)
        if held:
            for o, osbs in held:
                p2_store(states[n_s - 1], o, osbs)


def build_nc(n_s=B_PER_CORE, C=C_FULL, N=H_FULL * W_FULL, **kwargs):
    nc = bacc.Bacc("TRN2", target_bir_lowering=False, debug=False)
    x = nc.dram_tensor("x", [n_s, C, N], F32, kind="ExternalInput").ap()
    gamma_b = nc.dram_tensor("gamma_b", [P, 1], F32, kind="ExternalInput").ap()
    out = nc.dram_tensor("out", [n_s, C, N], F32, kind="ExternalOutput").ap()
    with tile.TileContext(nc) as tc:
        emit_cam(tc, x, gamma_b, out, n_s, C, N, **kwargs)
    nc.compile()
    return nc


_CACHE = threading.Lock()
_NC = None


def _get_nc():
    global _NC
    with _CACHE:
        if _NC is None:
            _NC = build_nc()
    return _NC


def run_spmd(x, gamma, **kwargs):
    """Shard inputs over 8 cores, run, gather. Returns (output, BassKernelResults)."""
    x = np.ascontiguousarray(np.asarray(x), dtype=np.float32)
    assert x.shape == (B_FULL, C_FULL, H_FULL, W_FULL), x.shape
    n = H_FULL * W_FULL
    xs = x.reshape(B_FULL, C_FULL, n)
    gb = np.full((P, 1), np.float32(np.asarray(gamma)), dtype=np.float32)
    in_maps = [
        {"x": xs[c * B_PER_CORE:(c + 1) * B_PER_CORE], "gamma_b": gb}
        for c in range(N_CORES)
    ]
    nc = _get_nc()
    res = run_bass_kernel_spmd(nc, in_maps, core_ids=list(range(N_CORES)), **kwargs)
    outs = np.stack([res.results[c]["out"] for c in range(N_CORES)])
    full = outs.reshape(B_FULL, C_FULL, H_FULL, W_FULL).astype(np.float32, copy=False)
    return full, res


def kernel(x, gamma):
    out, _ = run_spmd(x, gamma)
    return out
